# revision 1
# baseline (speedup 1.0000x reference)
"""Multi-head attention (4x2048x1024, 16 heads) on 8 TRN2 NeuronCores.

Sharding: core c handles batch c//2, query seq-half c%2 (1024 queries).
Each core computes QKV projection for its own seq half plus K/V for the
peer half (redundant compute instead of a 2-rank collective), full
attention for all 16 heads over its 1024 queries x 2048 keys, and the
output projection. Outputs are disjoint -> no collectives; host concats.

Host passes transposed (d-major) shards with the core's own seq-half
first, so the SPMD graph is identical on every core.
"""

import numpy as np

import concourse.mybir as mybir
import concourse.tile as tile
from concourse import bacc
from concourse.bass_utils import run_bass_kernel_spmd
FP32 = mybir.dt.float32
BF16 = mybir.dt.bfloat16

DIM = 1024
HEADS = 16
HD = 64
AUG = HD + 1  # V columns per head + ones column for sum-exp
SCALE = DIM ** -0.5
SEQ = 2048
NI = 1024  # queries per core
NJ = 2048  # keys per core
B = 4
N_CORES = 8
P = 128

TRACE = False
LAST_RESULTS = None
_NC_CACHE = None


def _build():
    nc = bacc.Bacc(
        "TRN2",
        target_bir_lowering=False,
        debug=False,
        enable_asserts=False,
        num_devices=N_CORES,
    )
    xT = nc.dram_tensor("xT", [DIM, NJ], FP32, kind="ExternalInput")
    wqkvT = nc.dram_tensor("wqkvT", [DIM, 3 * DIM], FP32, kind="ExternalInput")
    woutT = nc.dram_tensor("woutT", [DIM, DIM], FP32, kind="ExternalInput")
    bout = nc.dram_tensor("bout", [1, DIM], FP32, kind="ExternalInput")
    out = nc.dram_tensor("out", [NI, DIM], FP32, kind="ExternalOutput")

    ND = DIM // P  # 8 contraction tiles

    with tile.TileContext(nc) as tc:
        with (
            tc.tile_pool(name="persist", bufs=1) as persist,
            tc.tile_pool(name="stage", bufs=3) as stage,
            tc.tile_pool(name="wpool", bufs=9) as wpool,
            tc.tile_pool(name="sb", bufs=3) as sb,
            tc.tile_pool(name="small", bufs=3) as small,
            tc.tile_pool(name="ps", bufs=4, space="PSUM") as psp,
        ):
            xpool_cm = tc.tile_pool(name="xpool", bufs=1)
            xpool = xpool_cm.__enter__()

            # ---- bias broadcast [1,1024] -> [128,1024]
            bias_sb = small.tile([1, DIM], FP32, tag="bias", name="bias", bufs=1)
            nc.sync.dma_start(out=bias_sb, in_=bout.ap())
            bias_bc = small.tile([P, DIM], FP32, tag="biasbc", name="biasbc", bufs=1)
            nc.gpsimd.partition_broadcast(bias_bc, bias_sb)

            def load_w_group(src_ap, ebase):
                """Load+cast 8 weight tiles [128 d, 1024 e] for one group."""
                grp = []
                for dt in range(ND):
                    ws = stage.tile([P, DIM], FP32, tag="stg", name="stg")
                    nc.sync.dma_start(
                        out=ws,
                        in_=src_ap[dt * P:(dt + 1) * P, ebase:ebase + DIM],
                    )
                    wb = wpool.tile([P, DIM], BF16, tag="wbf", name="wbf")
                    nc.vector.tensor_copy(wb, ws)
                    grp.append(wb)
                return grp

            def load_x_half(xbf, half):
                for dt in range(ND):
                    xs = stage.tile([P, DIM], FP32, tag="stg", name="stg")
                    nc.sync.dma_start(
                        out=xs,
                        in_=xT.ap()[dt * P:(dt + 1) * P,
                                    half * DIM:(half + 1) * DIM],
                    )
                    dst = xbf[dt][:, half * DIM:(half + 1) * DIM]
                    if (dt + half) % 2 == 0:
                        nc.vector.tensor_copy(dst, xs)
                    else:
                        nc.scalar.copy(dst, xs)

            # ---- Q first (own-half x + Q weights only) so PE starts early
            xbf = [xpool.tile([P, NJ], BF16, tag=f"xbf{dt}", name=f"xbf{dt}")
                   for dt in range(ND)]
            wq = []
            for dt in range(ND):
                ws = stage.tile([P, DIM], FP32, tag="stg", name="stg")
                nc.sync.dma_start(
                    out=ws, in_=wqkvT.ap()[dt * P:(dt + 1) * P, 0:DIM])
                wb = wpool.tile([P, DIM], BF16, tag="wbf", name="wbf")
                nc.vector.tensor_copy(wb, ws)
                wq.append(wb)
                xs = stage.tile([P, DIM], FP32, tag="stg", name="stg")
                nc.sync.dma_start(
                    out=xs, in_=xT.ap()[dt * P:(dt + 1) * P, 0:DIM])
                nc.scalar.copy(xbf[dt][:, 0:DIM], xs)

            qt = [persist.tile([P, NI], BF16, tag=f"qt{e}", name=f"qt{e}")
                  for e in range(8)]
            kt = [persist.tile([P, NJ], BF16, tag=f"kt{e}", name=f"kt{e}")
                  for e in range(8)]

            def qk_proj(wg, tiles, chunks):
                # e-major out [e, n]; stationary (weight) reused per chunk set
                for et in range(8):
                    pss = {ch: psp.tile([P, DIM], FP32, tag="ps", name="ps")
                           for ch in chunks}
                    for dt in range(ND):
                        for ch in chunks:
                            for sc in range(2):
                                nb = ch * DIM + sc * 512
                                nc.tensor.matmul(
                                    pss[ch][:, sc * 512:(sc + 1) * 512],
                                    wg[dt][:, et * P:(et + 1) * P],
                                    xbf[dt][:, nb:nb + 512],
                                    start=(dt == 0),
                                    stop=(dt == ND - 1),
                                )
                    for ch in chunks:
                        dst = tiles[et][:, ch * DIM:(ch + 1) * DIM]
                        if (et + ch) % 2 == 0:
                            nc.vector.tensor_copy(dst, pss[ch])
                        else:
                            nc.scalar.copy(dst, pss[ch])

            qk_proj(wq, qt, [0])

            load_x_half(xbf, 1)
            wk = load_w_group(wqkvT.ap(), DIM)
            qk_proj(wk, kt, [0, 1])

            # ---- V projection: out n-major [n, e], scattered into 65-wide
            # per-head blocks with a ones column at offset 64 (sum-exp trick)
            vaug = [persist.tile([P, HEADS * AUG], BF16, tag=f"va{j}", name=f"va{j}")
                    for j in range(16)]
            for jt in range(16):
                v3 = vaug[jt].rearrange("p (h c) -> p h c", c=AUG)
                nc.vector.memset(v3[:, :, HD:AUG], 1.0)
            wv = load_w_group(wqkvT.ap(), 2 * DIM)

            def v_proj(jts):
                for jt in jts:
                    ps = psp.tile([P, DIM], FP32, tag="ps", name="ps")
                    for dt in range(ND):
                        for sc in range(2):  # e-chunks of 512 = 8 heads each
                            nc.tensor.matmul(
                                ps[:, sc * 512:(sc + 1) * 512],
                                xbf[dt][:, jt * P:(jt + 1) * P],
                                wv[dt][:, sc * 512:(sc + 1) * 512],
                                start=(dt == 0),
                                stop=(dt == ND - 1),
                            )
                    vsrc = ps.rearrange("p (h c) -> p h c", c=HD)
                    vdst = vaug[jt].rearrange("p (h c) -> p h c", c=AUG)[:, :, 0:HD]
                    nc.scalar.copy(vdst, vsrc)

            v_proj(range(16))

            # prefetch out-projection weights; DMAs+casts run during attention
            wo = load_w_group(woutT.ap(), 0)

            xpool_cm.__exit__(None, None, None)
            norm_cm = tc.tile_pool(name="norm", bufs=2)
            norm = norm_cm.__enter__()

            # ---- attention, head pairs (2hp, 2hp+1) share e-tile hp.
            # Pair 0 is emitted before the peer-half K/V projections: its
            # jt 0-7 (own half) can start as soon as own-half proj is done,
            # so the ScalarE exp stream starts ~60us earlier; the peer
            # projections then overlap pair 0's jt 8-15 dependencies.
            aot = [persist.tile([P, NI], BF16, tag=f"ao{e}", name=f"ao{e}")
                   for e in range(8)]

            def attention_pair(hp, jts=range(16), avs=None):
                if avs is None:
                    avA = psp.tile([AUG, NI], FP32, tag="ps", name="av")
                    avB = psp.tile([AUG, NI], FP32, tag="ps", name="av")
                else:
                    avA, avB = avs
                for jt in jts:
                    dA = psp.tile([P, NI], FP32, tag="ps", name="dots")
                    dB = psp.tile([P, NI], FP32, tag="ps", name="dots")
                    jsl = slice(jt * P, (jt + 1) * P)
                    # alternate row groups (A at rows 0-63, B at 64-127) so
                    # consecutive dots run concurrently on disjoint sub-arrays
                    for ic in range(2):
                        isl = slice(ic * 512, (ic + 1) * 512)
                        nc.tensor.matmul(
                            dA[:, isl], kt[hp][0:HD, jsl], qt[hp][0:HD, isl],
                            start=True, stop=True,
                        )
                        nc.tensor.matmul(
                            dB[:, isl], kt[hp][HD:P, jsl], qt[hp][HD:P, isl],
                            start=True, stop=True,
                        )
                    eA = sb.tile([P, NI], BF16, tag="expT", name="expT", bufs=3)
                    nc.scalar.activation(eA, dA, mybir.ActivationFunctionType.Exp,
                                         scale=SCALE)
                    eB = sb.tile([P, NI], BF16, tag="expT", name="expT", bufs=3)
                    nc.scalar.activation(eB, dB, mybir.ActivationFunctionType.Exp,
                                         scale=SCALE)
                    first, last = jt == 0, jt == 15
                    for av, e_t, head in ((avA, eA, 2 * hp), (avB, eB, 2 * hp + 1)):
                        for ic in range(2):
                            isl = slice(ic * 512, (ic + 1) * 512)
                            nc.tensor.matmul(
                                av[:, isl],
                                vaug[jt][:, head * AUG:(head + 1) * AUG],
                                e_t[:, isl],
                                start=first, stop=last,
                            )
                    # PE "heater" burst: a short run of full 128x128 matmuls
                    # with no concurrent PSUM readers restores the PE from the
                    # degraded state that sustained ACT-PSUM-read contention
                    # latches it into (measured: bursts net ~25% faster
                    # attention despite the wasted work).
                    if jt in (7, 15):
                        for _ in range(4):
                            pb = psp.tile([P, NI], FP32, tag="ps", name="heat")
                            for sc in range(2):
                                nc.tensor.matmul(
                                    pb[:, sc * 512:(sc + 1) * 512],
                                    kt[hp][:, 0:P],
                                    qt[hp][:, 0:512],
                                    start=True, stop=True,
                                )
                    # PE "heater" burst: a short run of full 128x128 matmuls
                    # with no concurrent PSUM readers restores the PE from the
                    # degraded state that sustained ACT-PSUM-read contention
                    # latches it into (measured: bursts net ~25% faster
                    # attention despite the wasted work).
                    if jt in (7, 15):
                        for _ in range(4):
                            pb = psp.tile([P, NI], FP32, tag="ps", name="heat")
                            for sc in range(2):
                                nc.tensor.matmul(
                                    pb[:, sc * 512:(sc + 1) * 512],
                                    kt[hp][:, 0:P],
                                    qt[hp][:, 0:512],
                                    start=True, stop=True,
                                )
                if 15 not in jts:
                    return (avA, avB)
                # per-pair softmax normalization, overlapped with the next
                # pair's attention. Sum-exp rows gathered at partitions 0/32
                # (32-aligned as DVE requires), one reciprocal per pair;
                # partition_broadcast only accepts base-0 inputs, so head B's
                # row goes through a base-0 temp.
                g = norm.tile([33, NI], FP32, tag="g", name="g")
                nc.vector.tensor_copy(aot[hp][0:HD, :], avA[0:HD, :])
                nc.vector.tensor_copy(aot[hp][HD:P, :], avB[0:HD, :])
                nc.vector.tensor_copy(g[0:1, :], avA[HD:AUG, :])
                nc.vector.tensor_copy(g[32:33, :], avB[HD:AUG, :])
                rp = norm.tile([33, NI], FP32, tag="rp", name="rp")
                nc.vector.reciprocal(rp, g)  # rows 1..31 junk, unused
                rbA = norm.tile([P, NI], FP32, tag="rb", name="rb")
                nc.gpsimd.partition_broadcast(rbA, rp[0:1, :])
                nc.gpsimd.tensor_mul(
                    aot[hp][0:HD, :], aot[hp][0:HD, :], rbA[0:HD, :])
                tb = norm.tile([1, NI], FP32, tag="tb", name="tb")
                nc.vector.tensor_copy(tb, rp[32:33, :])
                rbB = norm.tile([P, NI], FP32, tag="rb", name="rb")
                nc.gpsimd.partition_broadcast(rbB, tb)
                nc.gpsimd.tensor_mul(
                    aot[hp][HD:P, :], aot[hp][HD:P, :], rbB[HD:P, :])

            for hp in range(8):
                attention_pair(hp)
            # prefetch out-projection weights; DMAs+casts run during attention
            wo = load_w_group(woutT.ap(), 0)

            norm_cm.__exit__(None, None, None)

            # ---- output projection + bias
            for it in range(8):
                ps = psp.tile([P, DIM], FP32, tag="ps", name="ps")
                for et in range(8):
                    for fc in range(2):
                        fsl = slice(fc * 512, (fc + 1) * 512)
                        nc.tensor.matmul(
                            ps[:, fsl],
                            aot[et][:, it * P:(it + 1) * P],
                            wo[et][:, fsl],
                            start=(et == 0),
                            stop=(et == 7),
                        )
                osb = sb.tile([P, DIM], FP32, tag="outsb", name="outsb", bufs=2)
                nc.vector.tensor_add(osb, ps, bias_bc)
                nc.sync.dma_start(out=out.ap()[it * P:(it + 1) * P, :], in_=osb)

    nc.compile()
    return nc


def _get_nc():
    global _NC_CACHE
    if _NC_CACHE is None:
        _NC_CACHE = _build()
    return _NC_CACHE


def kernel(x, w_qkv, w_out, b_out):
    global LAST_RESULTS
    x = np.asarray(x, dtype=np.float32)
    w_qkv = np.asarray(w_qkv, dtype=np.float32)
    w_out = np.asarray(w_out, dtype=np.float32)
    b_out = np.asarray(b_out, dtype=np.float32)

    nc = _get_nc()

    wqkvT = np.ascontiguousarray(w_qkv.T)
    woutT = np.ascontiguousarray(w_out.T)
    brow = np.ascontiguousarray(b_out.reshape(1, DIM))

    in_maps = []
    for c in range(N_CORES):
        b, h = divmod(c, 2)
        own = x[b, h * NI:(h + 1) * NI, :]
        peer = x[b, (1 - h) * NI:(2 - h) * NI, :]
        xTc = np.ascontiguousarray(np.concatenate([own, peer], axis=0).T)
        in_maps.append({
            "xT": xTc,
            "wqkvT": wqkvT,
            "woutT": woutT,
            "bout": brow,
        })

    res = run_bass_kernel_spmd(
        nc, in_maps, core_ids=list(range(N_CORES)), trace=TRACE
    )
    LAST_RESULTS = res

    out = np.empty((B, SEQ, DIM), dtype=np.float32)
    for c in range(N_CORES):
        b, h = divmod(c, 2)
        out[b, h * NI:(h + 1) * NI, :] = res.results[c]["out"]
    return out

